# revision 4
# baseline (speedup 1.0000x reference)
"""Multi-head attention (B=4, N=2048, C=1024, H=16, D=64) on 8 TRN2 NeuronCores.

Sharding: core c handles batch b = c//2 and head-group g = c%2 (8 heads = 512
dims). Each core computes qkv projection, attention, and a partial output
projection for its head slice; the host sums the two partials per batch and
adds the proj bias.

v2 design (all matmuls bf16; tolerance budget 2e-2 allows it):
  - x / W_qkv / W_proj cast to bf16 on host; x^T produced by DMA-transpose
    (no PE transposes).
  - qkv projection: standard 128x128x512 bf16 matmuls (FWL hides ldweights).
  - attention per (slab=512 q, half=2 head-pairs, chunk=128 k):
      S^T: row-packed pairs (two 64-contraction matmuls at tile_position
           (0,0)/(64,0) run concurrently in the PE array)
      exp: ScalarE ACT exp for most chunks; VectorE Schraudolph fast-exp
           (one tensor_scalar f32->int16, bitcast bf16) for ~37% of chunks
      PV:  col-packed pairs (two 64-col matmuls at (0,0)/(0,64))
      denominators: quad-packed M=1 ones-matmuls at (0,{0,32,64,96})
      normalize: reciprocal + PE broadcast, fused into the PSUM->SBUF
           evacuation (scalar_tensor_tensor multiply)
  - output projection per slab, overlapped by the scheduler.
fp32r cannot col-tile (ISA: col_grp must be 0xf for fp32 HIGH) - bf16 is what
makes the PV/denominator packing legal.
"""

from contextlib import ExitStack

import ml_dtypes
import numpy as np

import concourse.bass as bass
import concourse.tile as tile
from concourse import bacc, mybir
from concourse.bass_utils import run_bass_kernel_spmd
from concourse.masks import make_identity

P = 128
N = 2048          # tokens per batch
C = 1024          # model dim
DC = 512          # head dims per core (8 heads x 64)
NSLABS = N // 512
F32 = mybir.dt.float32
BF16 = mybir.dt.bfloat16
I16 = mybir.dt.int16

# Schraudolph fast-exp in bf16-bit space, softmax scale 1/8 folded in:
# bf16_bits = round(logit * 0.125 * 2^7/ln2 + (127*2^7 - 486411/65536))
SCH_A = 12102203.161561485 / 65536.0 * 0.125
SCH_B = 1064866805.0 / 65536.0


def build_program(trace_label: str = "attn2"):
    nc = bacc.Bacc("TRN2", target_bir_lowering=False, name=trace_label)
    x_d = nc.dram_tensor("x", [N, C], BF16, kind="ExternalInput").ap()
    wqkv_d = nc.dram_tensor("wqkv", [C, 3 * DC], BF16, kind="ExternalInput").ap()
    wproj_d = nc.dram_tensor("wproj", [DC, C], BF16, kind="ExternalInput").ap()
    out_d = nc.dram_tensor("out", [N, C], F32, kind="ExternalOutput").ap()

    with tile.TileContext(nc) as tc, ExitStack() as ctx:
        _emit(ctx, tc, x_d, wqkv_d, wproj_d, out_d)
    nc.compile()
    return nc


def _emit(ctx, tc, x_d, wqkv_d, wproj_d, out_d):
    nc = tc.nc
    MULT = mybir.AluOpType.mult
    ADD = mybir.AluOpType.add
    BYPASS = mybir.AluOpType.bypass

    const = ctx.enter_context(tc.tile_pool(name="const", bufs=1))
    ident32 = const.tile([P, P], F32, tag="ident32")
    make_identity(nc, ident32)
    identb = const.tile([P, P], BF16, tag="identb")
    nc.vector.tensor_copy(identb[:], ident32[:])
    onesq = const.tile([P, 1], BF16, tag="onesq")
    nc.any.memset(onesq[:], 1.0)
    onescol = const.tile([P, 64], BF16, tag="onescol")
    nc.any.memset(onescol[:], 1.0)

    persist = ctx.enter_context(tc.tile_pool(name="persist", bufs=1))
    xT = persist.tile([P, 8, N], BF16, tag="xT")        # [c%128, c//128, n]
    wq = persist.tile([P, 8, 3 * DC], BF16, tag="wq")   # [c%128, c//128, col]
    wp = persist.tile([P, 4, C], BF16, tag="wp")        # [d%128, d//128, c]
    qT = persist.tile([P, 4, N], BF16, tag="qT")        # [d%128, pair, n]
    kT = persist.tile([P, 4, N], BF16, tag="kT")
    va = persist.tile([P, 16, DC], BF16, tag="va")      # [n%128, n//128, d]
    aT = persist.tile([P, 4, N], BF16, tag="aT")        # attn out^T

    # ---------------- DMAs + PE warmup ----------------
    # x^T via xbar DMA-transpose, in 512-token pieces so the first qkv
    # groups can start before the whole transpose lands.
    for ns in range(NSLABS):
        for cc in range(8):
            nc.sync.dma_start_transpose(
                xT[:, cc, ns * 512:(ns + 1) * 512],
                x_d[ns * 512:(ns + 1) * 512, cc * P:(cc + 1) * P],
            )
    for cc in range(8):
        nc.scalar.dma_start(wq[:, cc, :], wqkv_d[cc * P:(cc + 1) * P, :])
    for dc in range(4):
        nc.scalar.dma_start(wp[:, dc, :], wproj_d[dc * P:(dc + 1) * P, :])

    with tc.tile_pool(name="ps_warm", bufs=1, space="PSUM") as ps_warm:
        warm = ps_warm.tile([P, P], F32, tag="warm")
        for _ in range(72):
            nc.tensor.matmul(warm[:], identb[:], identb[:])

    # ---------------- phase 1: qkv projection ----------------
    evac_flip = [0]

    def evac(dst, src):
        if evac_flip[0] % 2 == 0:
            nc.vector.tensor_copy(dst, src)
        else:
            nc.scalar.copy(dst, src)
        evac_flip[0] += 1

    with tc.tile_pool(name="ps1", bufs=3, space="PSUM") as ps1:
        def kq_group(dst, colbase, dc, ns):
            ps = ps1.tile([P, 512], F32, tag="ps1")
            for cc in range(8):
                nc.tensor.matmul(
                    ps[:],
                    wq[:, cc, colbase + dc * P:colbase + (dc + 1) * P],
                    xT[:, cc, ns * 512:(ns + 1) * 512],
                    start=(cc == 0), stop=(cc == 7),
                )
            evac(dst[:, dc, ns * 512:(ns + 1) * 512], ps[:])

        for ns in range(NSLABS):
            for dc in range(4):
                kq_group(kT, DC, dc, ns)
        for nck in range(16):
            ps = ps1.tile([P, 512], F32, tag="ps1")
            for cc in range(8):
                nc.tensor.matmul(
                    ps[:],
                    xT[:, cc, nck * P:(nck + 1) * P],
                    wq[:, cc, 2 * DC:3 * DC],
                    start=(cc == 0), stop=(cc == 7),
                )
            evac(va[:, nck, :], ps[:])
        for ns in range(NSLABS):
            for dc in range(4):
                kq_group(qT, 0, dc, ns)

    # ---------------- phase 2: attention + proj ----------------
    with tc.tile_pool(name="st", bufs=2, space="PSUM") as st_pool, \
         tc.tile_pool(name="pvproj", bufs=2, space="PSUM") as pv_pool, \
         tc.tile_pool(name="dn", bufs=1, space="PSUM") as dn_pool, \
         tc.tile_pool(name="bc", bufs=1, space="PSUM") as bc_pool, \
         tc.tile_pool(name="epool", bufs=6) as epool, \
         tc.tile_pool(name="rc32", bufs=2) as rc32_pool, \
         tc.tile_pool(name="rcb", bufs=2) as rcb_pool, \
         tc.tile_pool(name="bcs", bufs=2) as bcs_pool, \
         tc.tile_pool(name="oproj", bufs=2) as opool:

        for s in range(NSLABS):
            for half in range(2):
                pvt = [
                    pv_pool.tile([P, 512], F32, tag="pv",
                                 name=f"pv{s}_{half}_{lp}")
                    for lp in range(2)
                ]
                dnt = dn_pool.tile([P, 512], F32, tag="dn",
                                   name=f"dn{s}_{half}")

                def flush(item):
                    cck, es = item
                    for lp in range(2):
                        for sub in range(2):
                            o = 64 * sub
                            h = 4 * half + 2 * lp + sub
                            nc.tensor.matmul(
                                pvt[lp][o:o + 64, :],
                                va[:, cck, 64 * h:64 * h + 64],
                                es[lp][:, sub, :],
                                start=(cck == 0), stop=(cck == 15),
                                tile_position=(0, o),
                            )
                    for j in range(4):
                        lp, sub = j // 2, j % 2
                        nc.tensor.matmul(
                            dnt[32 * j:32 * j + 1, :],
                            onesq[:, :],
                            es[lp][:, sub, :],
                            start=(cck == 0), stop=(cck == 15),
                            tile_position=(0, 32 * j),
                        )

                pending = []
                for ck in range(16):
                    es = []
                    for lp in range(2):
                        p = 2 * half + lp
                        st = st_pool.tile([P, 2, 512], F32, tag="st")
                        for sub in range(2):
                            o = 64 * sub
                            nc.tensor.matmul(
                                st[:, sub, :],
                                kT[o:o + 64, p, ck * P:(ck + 1) * P],
                                qT[o:o + 64, p, s * 512:(s + 1) * 512],
                                tile_position=(o, 0),
                            )
                        e = epool.tile([P, 2, 512], BF16, tag="e")
                        # ~37% of exps ride VectorE via Schraudolph
                        if lp == 1 and ck % 4 != 3:
                            nc.vector.tensor_scalar(
                                e.bitcast(I16)[:], st[:], SCH_A, SCH_B,
                                MULT, ADD,
                            )
                        else:
                            nc.scalar.activation(
                                e[:], st[:],
                                mybir.ActivationFunctionType.Exp, scale=0.125,
                            )
                        es.append(e)
                    pending.append((ck, es))
                    if len(pending) > 2:
                        flush(pending.pop(0))
                for item in pending:
                    flush(item)

                # normalize: recip of the 4 denominator rows, PE-broadcast,
                # fused multiply during PSUM->SBUF evacuation.
                rc32 = rc32_pool.tile([P, 512], F32, tag="rc32",
                                      name=f"rc32_{s}_{half}")
                nc.vector.reciprocal_approx_fast(rc32[0:97, :], dnt[0:97, :])
                rc = rcb_pool.tile([P, 512], BF16, tag="rc",
                                   name=f"rc_{s}_{half}")
                nc.vector.tensor_copy(rc[0:97, :], rc32[0:97, :])
                for lp in range(2):
                    bct = bc_pool.tile([P, 512], F32, tag="bc",
                                       name=f"bc{s}_{half}_{lp}")
                    for sub in range(2):
                        j = 2 * lp + sub
                        nc.tensor.matmul(
                            bct[64 * sub:64 * sub + 64, :],
                            onescol[32 * j:32 * j + 1, :],
                            rc[32 * j:32 * j + 1, :],
                            tile_position=(32 * j, 64 * sub),
                        )
                    # DVE can't read two PSUM operands in one op; stage the
                    # broadcast in SBUF (on ScalarE, which has tail slack)
                    bcs = bcs_pool.tile([P, 512], F32, tag="bcs",
                                        name=f"bcs{s}_{half}_{lp}")
                    nc.scalar.copy(bcs[:], bct[:])
                    p = 2 * half + lp
                    nc.vector.scalar_tensor_tensor(
                        aT[:, p, s * 512:(s + 1) * 512],
                        pvt[lp][:], 0.0, bcs[:], BYPASS, MULT,
                    )

            # ---- output projection for this slab ----
            for i in range(4):
                nck = 4 * s + i
                for ct in range(2):
                    pp = pv_pool.tile([P, 512], F32, tag="pv",
                                      name=f"proj{nck}_{ct}")
                    for dc in range(4):
                        nc.tensor.matmul(
                            pp[:],
                            aT[:, dc, nck * P:(nck + 1) * P],
                            wp[:, dc, ct * 512:(ct + 1) * 512],
                            start=(dc == 0), stop=(dc == 3),
                        )
                    ot = opool.tile([P, 512], F32, tag="ot")
                    evac(ot[:], pp[:])
                    nc.sync.dma_start(
                        out_d[nck * P:(nck + 1) * P, ct * 512:(ct + 1) * 512],
                        ot[:],
                    )


def shard_inputs(x, W_qkv, W_proj):
    """Full inputs -> 8 per-core in_maps. Core c: batch c//2, head-group c%2."""
    x = np.asarray(x, dtype=np.float32)
    W_qkv = np.asarray(W_qkv, dtype=np.float32)
    W_proj = np.asarray(W_proj, dtype=np.float32)
    bf = ml_dtypes.bfloat16
    in_maps = []
    for core in range(8):
        b, g = core // 2, core % 2
        cols = slice(g * DC, (g + 1) * DC)
        w = np.concatenate(
            [W_qkv[:, 0:C][:, cols], W_qkv[:, C:2 * C][:, cols],
             W_qkv[:, 2 * C:3 * C][:, cols]],
            axis=1,
        )
        in_maps.append({
            "x": np.ascontiguousarray(x[b]).astype(bf),
            "wqkv": np.ascontiguousarray(w).astype(bf),
            "wproj": np.ascontiguousarray(W_proj[g * DC:(g + 1) * DC, :]).astype(bf),
        })
    return in_maps


def unshard_output(results, b_proj):
    b_proj = np.asarray(b_proj, dtype=np.float32)
    out = np.empty((4, N, C), dtype=np.float32)
    for b in range(4):
        out[b] = results[2 * b]["out"] + results[2 * b + 1]["out"] + b_proj[None, :]
    return out


_NC_CACHE = []


def kernel(x, W_qkv, W_proj, b_proj, trace=False):
    in_maps = shard_inputs(x, W_qkv, W_proj)
    if not _NC_CACHE:
        _NC_CACHE.append(build_program())
    nc = _NC_CACHE[0]
    res = run_bass_kernel_spmd(nc, in_maps, core_ids=list(range(8)), trace=trace)
    out = unshard_output(res.results, b_proj)
    if trace:
        return out, res
    return out


# revision 6
# speedup vs baseline: 1.1089x; 1.1089x over previous
"""Multi-head attention (B=4, N=2048, C=1024, H=16, D=64) on 8 TRN2 NeuronCores.

Sharding: core c handles batch b = c//2 and head-group g = c%2 (8 heads = 512
dims). Each core computes qkv projection, attention, and a partial output
projection for its head slice; the host sums the two partials per batch and
adds the proj bias.

v3 design (all matmuls bf16; the 2e-2 tolerance budget allows it):
  - host passes x pre-transposed (xT [C, N]) and all weights in bf16; no
    device-side transposes at all.
  - qkv projection: pairs of accumulation groups interleaved so each
    ldweights hides under the other group's matmul.
  - attention, pair-major, in 2-chunk groups (runs of 4 same-shape matmuls
    pack in the PE array):
      S^T: row-packed pairs (two 64-contraction matmuls at tile_position
           (0,0)/(64,0) run concurrently)
      exp: ScalarE ACT exp for 9/16 chunks; VectorE Schraudolph fast-exp
           (one tensor_scalar f32->int16, bitcast bf16) for 7/16
      PV:  col-packed pairs (two 64-col matmuls at (0,0)/(0,64)) into one
           PSUM bank per head-pair
      denominators: M=1 ones-matmuls, even chunks -> rows {0,32}, odd ->
           {64,96}, so each 2-chunk group is one quad of concurrent tiles
      normalize: off the critical path - pv evacuates unnormalized (frees
           the bank), then recip + PE broadcast + in-place SBUF multiply
  - PSUM: st pool 3x2 banks (elasticity so S never waits on exp latency),
    pv 1, dn 1; proj and the recip-broadcast borrow st-pool tiles.
fp32r cannot col-tile (ISA: col_grp must be 0xf for fp32 HIGH) - bf16 is
what makes the PV/denominator packing legal.
"""

from contextlib import ExitStack

import ml_dtypes
import numpy as np

import concourse.bass as bass
import concourse.tile as tile
from concourse import bacc, mybir
from concourse.bass_utils import run_bass_kernel_spmd
from concourse.masks import make_identity

P = 128
N = 2048          # tokens per batch
C = 1024          # model dim
DC = 512          # head dims per core (8 heads x 64)
NSLABS = N // 512
F32 = mybir.dt.float32
BF16 = mybir.dt.bfloat16
I16 = mybir.dt.int16

# Schraudolph fast-exp in bf16-bit space, softmax scale 1/8 folded in:
# bf16_bits = round(logit * 0.125 * 2^7/ln2 + (127*2^7 - 486411/65536))
SCH_A = 12102203.161561485 / 65536.0 * 0.125
SCH_B = 1064866805.0 / 65536.0
DVE_CKS = frozenset({0, 2, 5, 7, 9, 12, 14})  # 7/16 chunks exp'd on VectorE


def build_program(trace_label: str = "attn3"):
    nc = bacc.Bacc("TRN2", target_bir_lowering=False, name=trace_label)
    xT_d = nc.dram_tensor("xT", [C, N], BF16, kind="ExternalInput").ap()
    wqkv_d = nc.dram_tensor("wqkv", [C, 3 * DC], BF16, kind="ExternalInput").ap()
    wproj_d = nc.dram_tensor("wproj", [DC, C], BF16, kind="ExternalInput").ap()
    out_d = nc.dram_tensor("out", [N, C], F32, kind="ExternalOutput").ap()

    with tile.TileContext(nc) as tc, ExitStack() as ctx:
        _emit(ctx, tc, xT_d, wqkv_d, wproj_d, out_d)
    nc.compile()
    return nc


def _emit(ctx, tc, xT_d, wqkv_d, wproj_d, out_d):
    nc = tc.nc
    MULT = mybir.AluOpType.mult
    ADD = mybir.AluOpType.add
    BYPASS = mybir.AluOpType.bypass

    const = ctx.enter_context(tc.tile_pool(name="const", bufs=1))
    ident32 = const.tile([P, P], F32, tag="ident32")
    make_identity(nc, ident32)
    identb = const.tile([P, P], BF16, tag="identb")
    nc.vector.tensor_copy(identb[:], ident32[:])
    onesq = const.tile([P, 1], BF16, tag="onesq")
    nc.any.memset(onesq[:], 1.0)
    onescol = const.tile([P, 64], BF16, tag="onescol")
    nc.any.memset(onescol[:], 1.0)

    persist = ctx.enter_context(tc.tile_pool(name="persist", bufs=1))
    xT = persist.tile([P, 8, N], BF16, tag="xT")        # [c%128, c//128, n]
    wq = persist.tile([P, 8, 3 * DC], BF16, tag="wq")   # [c%128, c//128, col]
    wp = persist.tile([P, 4, C], BF16, tag="wp")        # [d%128, d//128, c]
    qT = persist.tile([P, 4, N], BF16, tag="qT")        # [d%128, pair, n]
    kT = persist.tile([P, 4, N], BF16, tag="kT")
    va = persist.tile([P, 16, DC], BF16, tag="va")      # [n%128, n//128, d]
    aT = persist.tile([P, 4, N], BF16, tag="aT")        # attn out^T

    # ---------------- DMAs + PE warmup ----------------
    for ns in range(NSLABS):
        for cc in range(8):
            nc.sync.dma_start(
                xT[:, cc, ns * 512:(ns + 1) * 512],
                xT_d[cc * P:(cc + 1) * P, ns * 512:(ns + 1) * 512],
            )
    for cc in range(8):
        nc.scalar.dma_start(wq[:, cc, :], wqkv_d[cc * P:(cc + 1) * P, :])
    for dc in range(4):
        nc.scalar.dma_start(wp[:, dc, :], wproj_d[dc * P:(dc + 1) * P, :])

    with tc.tile_pool(name="ps_warm", bufs=1, space="PSUM") as ps_warm:
        warm = ps_warm.tile([P, P], F32, tag="warm")
        for _ in range(48):
            nc.tensor.matmul(warm[:], identb[:], identb[:])

    # ---------------- phase 1: qkv projection ----------------
    evac_flip = [0]

    def evac(dst, src):
        if evac_flip[0] % 2 == 0:
            nc.vector.tensor_copy(dst, src)
        else:
            nc.scalar.copy(dst, src)
        evac_flip[0] += 1

    with tc.tile_pool(name="ps1", bufs=4, space="PSUM") as ps1:
        def kq_pair(dst, colbase, dc0, ns):
            # two accumulation groups interleaved: each ldweights hides
            # under the other group's streaming matmul
            psA = ps1.tile([P, 512], F32, tag="ps1")
            psB = ps1.tile([P, 512], F32, tag="ps1")
            for cc in range(8):
                for j, ps in ((0, psA), (1, psB)):
                    col = colbase + (dc0 + j) * P
                    nc.tensor.matmul(
                        ps[:],
                        wq[:, cc, col:col + P],
                        xT[:, cc, ns * 512:(ns + 1) * 512],
                        start=(cc == 0), stop=(cc == 7),
                    )
            evac(dst[:, dc0, ns * 512:(ns + 1) * 512], psA[:])
            evac(dst[:, dc0 + 1, ns * 512:(ns + 1) * 512], psB[:])

        def v_pair(nck0):
            psA = ps1.tile([P, 512], F32, tag="ps1")
            psB = ps1.tile([P, 512], F32, tag="ps1")
            for cc in range(8):
                for j, ps in ((0, psA), (1, psB)):
                    nck = nck0 + j
                    nc.tensor.matmul(
                        ps[:],
                        xT[:, cc, nck * P:(nck + 1) * P],
                        wq[:, cc, 2 * DC:3 * DC],
                        start=(cc == 0), stop=(cc == 7),
                    )
            evac(va[:, nck0, :], psA[:])
            evac(va[:, nck0 + 1, :], psB[:])

        for ns in range(NSLABS):
            for dc0 in (0, 2):
                kq_pair(kT, DC, dc0, ns)
        for nck0 in range(0, 16, 2):
            v_pair(nck0)
        for ns in range(NSLABS):
            for dc0 in (0, 2):
                kq_pair(qT, 0, dc0, ns)

    # ---------------- phase 2: attention + proj ----------------
    with tc.tile_pool(name="st", bufs=3, space="PSUM") as st_pool, \
         tc.tile_pool(name="pv", bufs=1, space="PSUM") as pv_pool, \
         tc.tile_pool(name="dn", bufs=1, space="PSUM") as dn_pool, \
         tc.tile_pool(name="epool", bufs=6) as epool, \
         tc.tile_pool(name="nrm", bufs=2) as nrm_pool, \
         tc.tile_pool(name="oproj", bufs=2) as opool:

        for s in range(NSLABS):
            for p in range(4):          # head pair (heads 2p, 2p+1)
                pv = pv_pool.tile([P, 512], F32, tag="pv", name=f"pv{s}_{p}")
                dn = dn_pool.tile([P, 512], F32, tag="dn", name=f"dn{s}_{p}")

                def flush(item):
                    # PV pair + denominator duo for one chunk; even chunks
                    # accumulate dn rows {0,32}, odd {64,96}, so adjacent
                    # chunks' duos form a 4-position concurrent quad.
                    cck, e = item
                    for sub in range(2):
                        o = 64 * sub
                        h = 2 * p + sub
                        nc.tensor.matmul(
                            pv[o:o + 64, :],
                            va[:, cck, 64 * h:64 * h + 64],
                            e[:, sub, :],
                            start=(cck == 0), stop=(cck == 15),
                            tile_position=(0, o),
                        )
                    ro = 64 * (cck % 2)
                    for sub in range(2):
                        r = ro + 32 * sub
                        nc.tensor.matmul(
                            dn[r:r + 1, :],
                            onesq[:, :],
                            e[:, sub, :],
                            start=(cck < 2), stop=(cck >= 14),
                            tile_position=(0, r),
                        )

                pending = []
                for g in range(8):       # 2-chunk groups
                    ges = []
                    for ck in (2 * g, 2 * g + 1):
                        st = st_pool.tile([P, 2, 512], F32, tag="st")
                        for sub in range(2):
                            o = 64 * sub
                            nc.tensor.matmul(
                                st[:, sub, :],
                                kT[o:o + 64, p, ck * P:(ck + 1) * P],
                                qT[o:o + 64, p, s * 512:(s + 1) * 512],
                                tile_position=(o, 0),
                            )
                        ges.append((ck, st))
                    for ck, st in ges:
                        e = epool.tile([P, 2, 512], BF16, tag="e")
                        if ck in DVE_CKS:
                            nc.vector.tensor_scalar(
                                e.bitcast(I16)[:], st[:], SCH_A, SCH_B,
                                MULT, ADD,
                            )
                        else:
                            nc.scalar.activation(
                                e[:], st[:],
                                mybir.ActivationFunctionType.Exp, scale=0.125,
                            )
                        pending.append((ck, e))
                    if len(pending) > 2:
                        flush(pending.pop(0))
                        flush(pending.pop(0))
                for item in pending:
                    flush(item)

                # tail: evacuate pv immediately (frees the bank), then
                # denominator combine + recip + PE broadcast + in-place
                # normalize, all off the pv critical path.
                aslc = aT[:, p, s * 512:(s + 1) * 512]
                nc.vector.tensor_copy(aslc, pv[:])
                dsb = nrm_pool.tile([33, 512], F32, tag="dsb",
                                    name=f"dsb{s}_{p}")
                nc.vector.tensor_copy(dsb[:], dn[64:97, :])
                dadd = nrm_pool.tile([33, 512], F32, tag="dadd",
                                     name=f"dadd{s}_{p}")
                nc.vector.scalar_tensor_tensor(
                    dadd[:], dn[0:33, :], 0.0, dsb[:], BYPASS, ADD,
                )
                rc32 = nrm_pool.tile([33, 512], F32, tag="rc32",
                                     name=f"rc32_{s}_{p}")
                nc.vector.reciprocal_approx_fast(rc32[:], dadd[:])
                rcb = nrm_pool.tile([33, 512], BF16, tag="rcb",
                                    name=f"rcb{s}_{p}")
                nc.vector.tensor_copy(rcb[:], rc32[:])
                bct = st_pool.tile([P, 2, 512], F32, tag="st",
                                   name=f"bc{s}_{p}")
                for sub in range(2):
                    nc.tensor.matmul(
                        bct[64 * sub:64 * sub + 64, 0, :],
                        onescol[32 * sub:32 * sub + 1, :],
                        rcb[32 * sub:32 * sub + 1, :],
                        tile_position=(32 * sub, 64 * sub),
                    )
                nc.vector.scalar_tensor_tensor(
                    aslc, aslc, 0.0, bct[:, 0, :], BYPASS, MULT,
                )

            # ---- output projection for this slab ----
            for i in range(4):
                nck = 4 * s + i
                for ct in range(2):
                    pp = st_pool.tile([P, 2, 512], F32, tag="st",
                                      name=f"proj{nck}_{ct}")
                    for dc in range(4):
                        nc.tensor.matmul(
                            pp[:, 0, :],
                            aT[:, dc, nck * P:(nck + 1) * P],
                            wp[:, dc, ct * 512:(ct + 1) * 512],
                            start=(dc == 0), stop=(dc == 3),
                        )
                    ot = opool.tile([P, 512], F32, tag="ot")
                    evac(ot[:], pp[:, 0, :])
                    nc.sync.dma_start(
                        out_d[nck * P:(nck + 1) * P, ct * 512:(ct + 1) * 512],
                        ot[:],
                    )


def shard_inputs(x, W_qkv, W_proj):
    """Full inputs -> 8 per-core in_maps. Core c: batch c//2, head-group c%2."""
    x = np.asarray(x, dtype=np.float32)
    W_qkv = np.asarray(W_qkv, dtype=np.float32)
    W_proj = np.asarray(W_proj, dtype=np.float32)
    bf = ml_dtypes.bfloat16
    in_maps = []
    for core in range(8):
        b, g = core // 2, core % 2
        cols = slice(g * DC, (g + 1) * DC)
        w = np.concatenate(
            [W_qkv[:, 0:C][:, cols], W_qkv[:, C:2 * C][:, cols],
             W_qkv[:, 2 * C:3 * C][:, cols]],
            axis=1,
        )
        in_maps.append({
            "xT": np.ascontiguousarray(x[b].T).astype(bf),
            "wqkv": np.ascontiguousarray(w).astype(bf),
            "wproj": np.ascontiguousarray(W_proj[g * DC:(g + 1) * DC, :]).astype(bf),
        })
    return in_maps


def unshard_output(results, b_proj):
    b_proj = np.asarray(b_proj, dtype=np.float32)
    out = np.empty((4, N, C), dtype=np.float32)
    for b in range(4):
        out[b] = results[2 * b]["out"] + results[2 * b + 1]["out"] + b_proj[None, :]
    return out


_NC_CACHE = []


def kernel(x, W_qkv, W_proj, b_proj, trace=False):
    in_maps = shard_inputs(x, W_qkv, W_proj)
    if not _NC_CACHE:
        _NC_CACHE.append(build_program())
    nc = _NC_CACHE[0]
    res = run_bass_kernel_spmd(nc, in_maps, core_ids=list(range(8)), trace=trace)
    out = unshard_output(res.results, b_proj)
    if trace:
        return out, res
    return out


# revision 9
# speedup vs baseline: 1.2128x; 1.0937x over previous
"""Multi-head attention (B=4, N=2048, C=1024, H=16, D=64) on 8 TRN2 NeuronCores.

Sharding: core c handles batch b = c//2 and head-group g = c%2 (8 heads = 512
dims). Each core computes qkv projection, attention, and a partial output
projection for its head slice; the host sums the two partials per batch and
adds the proj bias.

v3 design (all matmuls bf16; the 2e-2 tolerance budget allows it):
  - host passes x pre-transposed (xT [C, N]) and all weights in bf16; no
    device-side transposes at all.
  - qkv projection: pairs of accumulation groups interleaved so each
    ldweights hides under the other group's matmul.
  - attention, pair-major, in 2-chunk groups (runs of 4 same-shape matmuls
    pack in the PE array):
      S^T: row-packed pairs (two 64-contraction matmuls at tile_position
           (0,0)/(64,0) run concurrently)
      exp: ScalarE ACT exp for 9/16 chunks; VectorE Schraudolph fast-exp
           (one tensor_scalar f32->int16, bitcast bf16) for 7/16
      PV:  col-packed pairs (two 64-col matmuls at (0,0)/(0,64)) into one
           PSUM bank per head-pair
      denominators: M=1 ones-matmuls, even chunks -> rows {0,32}, odd ->
           {64,96}, so each 2-chunk group is one quad of concurrent tiles
      normalize: off the critical path - pv evacuates unnormalized (frees
           the bank), then recip + PE broadcast + in-place SBUF multiply
  - PSUM: st pool 3x2 banks (elasticity so S never waits on exp latency),
    pv 1, dn 1; proj and the recip-broadcast borrow st-pool tiles.
fp32r cannot col-tile (ISA: col_grp must be 0xf for fp32 HIGH) - bf16 is
what makes the PV/denominator packing legal.
"""

from contextlib import ExitStack

import ml_dtypes
import numpy as np

import concourse.bass as bass
import concourse.tile as tile
from concourse import bacc, mybir
from concourse.bass_utils import run_bass_kernel_spmd
from concourse.masks import make_identity

P = 128
N = 2048          # tokens per batch
C = 1024          # model dim
DC = 512          # head dims per core (8 heads x 64)
NSLABS = N // 512
F32 = mybir.dt.float32
BF16 = mybir.dt.bfloat16
I16 = mybir.dt.int16

# Schraudolph fast-exp in bf16-bit space, softmax scale 1/8 folded in:
# bf16_bits = round(logit * 0.125 * 2^7/ln2 + (127*2^7 - 486411/65536))
SCH_A = 12102203.161561485 / 65536.0 * 0.125
SCH_B = 1064866805.0 / 65536.0
DVE_CKS = frozenset({0, 2, 5, 7, 9, 12, 14})  # 7/16 chunks exp'd on VectorE


def build_program(trace_label: str = "attn3"):
    nc = bacc.Bacc("TRN2", target_bir_lowering=False, name=trace_label)
    xT_d = nc.dram_tensor("xT", [C, N], BF16, kind="ExternalInput").ap()
    wqkv_d = nc.dram_tensor("wqkv", [C, 3 * DC], BF16, kind="ExternalInput").ap()
    wproj_d = nc.dram_tensor("wproj", [DC, C], BF16, kind="ExternalInput").ap()
    out_d = nc.dram_tensor("out", [N, C], F32, kind="ExternalOutput").ap()

    with tile.TileContext(nc) as tc, ExitStack() as ctx:
        _emit(ctx, tc, xT_d, wqkv_d, wproj_d, out_d)
    nc.compile()
    return nc


def _emit(ctx, tc, xT_d, wqkv_d, wproj_d, out_d):
    nc = tc.nc
    MULT = mybir.AluOpType.mult
    ADD = mybir.AluOpType.add
    BYPASS = mybir.AluOpType.bypass

    const = ctx.enter_context(tc.tile_pool(name="const", bufs=1))
    ident32 = const.tile([P, P], F32, tag="ident32")
    make_identity(nc, ident32)
    identb = const.tile([P, P], BF16, tag="identb")
    nc.vector.tensor_copy(identb[:], ident32[:])
    onesq = const.tile([P, 1], BF16, tag="onesq")
    nc.any.memset(onesq[:], 1.0)
    onescol = const.tile([P, 64], BF16, tag="onescol")
    nc.any.memset(onescol[:], 1.0)

    persist = ctx.enter_context(tc.tile_pool(name="persist", bufs=1))
    xT = persist.tile([P, 8, N], BF16, tag="xT")        # [c%128, c//128, n]
    wq = persist.tile([P, 8, 3 * DC], BF16, tag="wq")   # [c%128, c//128, col]
    wp = persist.tile([P, 4, C], BF16, tag="wp")        # [d%128, d//128, c]
    qT = persist.tile([P, 4, N], BF16, tag="qT")        # [d%128, pair, n]
    kT = persist.tile([P, 4, N], BF16, tag="kT")
    va = persist.tile([P, 16, DC], BF16, tag="va")      # [n%128, n//128, d]
    aT = persist.tile([P, 4, N], BF16, tag="aT")        # attn out^T

    # ---------------- DMAs + PE warmup ----------------
    for ns in range(NSLABS):
        for cc in range(8):
            nc.sync.dma_start(
                xT[:, cc, ns * 512:(ns + 1) * 512],
                xT_d[cc * P:(cc + 1) * P, ns * 512:(ns + 1) * 512],
            )
    for cc in range(8):
        nc.scalar.dma_start(wq[:, cc, :], wqkv_d[cc * P:(cc + 1) * P, :])
    for dc in range(4):
        nc.scalar.dma_start(wp[:, dc, :], wproj_d[dc * P:(dc + 1) * P, :])

    with tc.tile_pool(name="ps_warm", bufs=1, space="PSUM") as ps_warm:
        warm = ps_warm.tile([P, P], F32, tag="warm")
        for _ in range(48):
            nc.tensor.matmul(warm[:], identb[:], identb[:])

    # ---------------- phase 1: qkv projection ----------------
    evac_flip = [0]

    def evac(dst, src):
        if evac_flip[0] % 2 == 0:
            nc.vector.tensor_copy(dst, src)
        else:
            nc.scalar.copy(dst, src)
        evac_flip[0] += 1

    with tc.tile_pool(name="ps1", bufs=4, space="PSUM") as ps1:
        def kq_pair(dst, colbase, dc0, ns):
            # two accumulation groups interleaved: each ldweights hides
            # under the other group's streaming matmul
            psA = ps1.tile([P, 512], F32, tag="ps1")
            psB = ps1.tile([P, 512], F32, tag="ps1")
            for cc in range(8):
                for j, ps in ((0, psA), (1, psB)):
                    col = colbase + (dc0 + j) * P
                    nc.tensor.matmul(
                        ps[:],
                        wq[:, cc, col:col + P],
                        xT[:, cc, ns * 512:(ns + 1) * 512],
                        start=(cc == 0), stop=(cc == 7),
                    )
            evac(dst[:, dc0, ns * 512:(ns + 1) * 512], psA[:])
            evac(dst[:, dc0 + 1, ns * 512:(ns + 1) * 512], psB[:])

        def v_pair(nck0):
            psA = ps1.tile([P, 512], F32, tag="ps1")
            psB = ps1.tile([P, 512], F32, tag="ps1")
            for cc in range(8):
                for j, ps in ((0, psA), (1, psB)):
                    nck = nck0 + j
                    nc.tensor.matmul(
                        ps[:],
                        xT[:, cc, nck * P:(nck + 1) * P],
                        wq[:, cc, 2 * DC:3 * DC],
                        start=(cc == 0), stop=(cc == 7),
                    )
            evac(va[:, nck0, :], psA[:])
            evac(va[:, nck0 + 1, :], psB[:])

        for ns in range(NSLABS):
            for dc0 in (0, 2):
                kq_pair(kT, DC, dc0, ns)
        for nck0 in range(0, 16, 2):
            v_pair(nck0)
        for ns in range(NSLABS):
            for dc0 in (0, 2):
                kq_pair(qT, 0, dc0, ns)

    # ---------------- phase 2: attention + proj ----------------
    with tc.tile_pool(name="st", bufs=3, space="PSUM") as st_pool, \
         tc.tile_pool(name="pv", bufs=1, space="PSUM") as pv_pool, \
         tc.tile_pool(name="dn", bufs=1, space="PSUM") as dn_pool, \
         tc.tile_pool(name="epool", bufs=6) as epool, \
         tc.tile_pool(name="nrm", bufs=2) as nrm_pool, \
         tc.tile_pool(name="oproj", bufs=2) as opool:

        deferred_bc = []

        def emit_bc(bs, bp, aslc, rcb):
            bct = st_pool.tile([P, 2, 512], F32, tag="st",
                               name=f"bc{bs}_{bp}")
            for sub in range(2):
                nc.tensor.matmul(
                    bct[64 * sub:64 * sub + 64, 0, :],
                    onescol[32 * sub:32 * sub + 1, :],
                    rcb[32 * sub:32 * sub + 1, :],
                    tile_position=(32 * sub, 64 * sub),
                )
            nc.vector.scalar_tensor_tensor(
                aslc, aslc, 0.0, bct[:, 0, :], BYPASS, MULT,
            )

        for s in range(NSLABS):
            for p in range(4):          # head pair (heads 2p, 2p+1)
                pv = pv_pool.tile([P, 512], F32, tag="pv", name=f"pv{s}_{p}")
                dn = dn_pool.tile([P, 512], F32, tag="dn", name=f"dn{s}_{p}")

                def flush(items):
                    # One 2-chunk group: all 4 PV matmuls back-to-back (runs
                    # of same-shape col-packed pairs), then the 4 denominator
                    # matmuls as one concurrent quad (even chunk -> rows
                    # {0,32}, odd -> {64,96}).
                    for cck, e in items:
                        for sub in range(2):
                            o = 64 * sub
                            h = 2 * p + sub
                            nc.tensor.matmul(
                                pv[o:o + 64, :],
                                va[:, cck, 64 * h:64 * h + 64],
                                e[:, sub, :],
                                start=(cck == 0), stop=(cck == 15),
                                tile_position=(0, o),
                            )
                    for cck, e in items:
                        ro = 64 * (cck % 2)
                        for sub in range(2):
                            r = ro + 32 * sub
                            nc.tensor.matmul(
                                dn[r:r + 1, :],
                                onesq[:, :],
                                e[:, sub, :],
                                start=(cck < 2), stop=(cck >= 14),
                                tile_position=(0, r),
                            )

                pending = []
                for g in range(8):       # 2-chunk groups
                    ges = []
                    for ck in (2 * g, 2 * g + 1):
                        st = st_pool.tile([P, 2, 512], F32, tag="st")
                        for sub in range(2):
                            o = 64 * sub
                            nc.tensor.matmul(
                                st[:, sub, :],
                                kT[o:o + 64, p, ck * P:(ck + 1) * P],
                                qT[o:o + 64, p, s * 512:(s + 1) * 512],
                                tile_position=(o, 0),
                            )
                        ges.append((ck, st))
                    for ck, st in ges:
                        e = epool.tile([P, 2, 512], BF16, tag="e")
                        if ck in DVE_CKS:
                            nc.vector.tensor_scalar(
                                e.bitcast(I16)[:], st[:], SCH_A, SCH_B,
                                MULT, ADD,
                            )
                        else:
                            nc.scalar.activation(
                                e[:], st[:],
                                mybir.ActivationFunctionType.Exp, scale=0.125,
                            )
                        pending.append((ck, e))
                    if len(pending) > 2:
                        flush([pending.pop(0), pending.pop(0)])
                    if deferred_bc:
                        # previous pair's PE broadcast, emitted here so the
                        # DVE recip chain it waits on overlaps this pair's
                        # chunks instead of stalling the PE queue
                        emit_bc(*deferred_bc.pop(0))
                flush(pending)
                pending = []

                # tail: evacuate pv immediately (frees the bank), then
                # denominator combine + recip on DVE; the PE broadcast +
                # in-place normalize are deferred into the next pair's
                # instruction stream.
                aslc = aT[:, p, s * 512:(s + 1) * 512]
                nc.vector.tensor_copy(aslc, pv[:])
                dsb = nrm_pool.tile([33, 512], F32, tag="dsb",
                                    name=f"dsb{s}_{p}")
                nc.vector.tensor_copy(dsb[:], dn[64:97, :])
                dadd = nrm_pool.tile([33, 512], F32, tag="dadd",
                                     name=f"dadd{s}_{p}")
                nc.vector.scalar_tensor_tensor(
                    dadd[:], dn[0:33, :], 0.0, dsb[:], BYPASS, ADD,
                )
                rc32 = nrm_pool.tile([33, 512], F32, tag="rc32",
                                     name=f"rc32_{s}_{p}")
                nc.vector.reciprocal_approx_fast(rc32[:], dadd[:])
                rcb = nrm_pool.tile([33, 512], BF16, tag="rcb",
                                    name=f"rcb{s}_{p}")
                nc.vector.tensor_copy(rcb[:], rc32[:])
                deferred_bc.append((s, p, aslc, rcb))

            while deferred_bc:
                emit_bc(*deferred_bc.pop(0))

            # ---- output projection for this slab ----
            for i in range(4):
                nck = 4 * s + i
                for ct in range(2):
                    pp = st_pool.tile([P, 2, 512], F32, tag="st",
                                      name=f"proj{nck}_{ct}")
                    for dc in range(4):
                        nc.tensor.matmul(
                            pp[:, 0, :],
                            aT[:, dc, nck * P:(nck + 1) * P],
                            wp[:, dc, ct * 512:(ct + 1) * 512],
                            start=(dc == 0), stop=(dc == 3),
                        )
                    ot = opool.tile([P, 512], F32, tag="ot")
                    evac(ot[:], pp[:, 0, :])
                    nc.sync.dma_start(
                        out_d[nck * P:(nck + 1) * P, ct * 512:(ct + 1) * 512],
                        ot[:],
                    )


def shard_inputs(x, W_qkv, W_proj):
    """Full inputs -> 8 per-core in_maps. Core c: batch c//2, head-group c%2."""
    x = np.asarray(x, dtype=np.float32)
    W_qkv = np.asarray(W_qkv, dtype=np.float32)
    W_proj = np.asarray(W_proj, dtype=np.float32)
    bf = ml_dtypes.bfloat16
    in_maps = []
    for core in range(8):
        b, g = core // 2, core % 2
        cols = slice(g * DC, (g + 1) * DC)
        w = np.concatenate(
            [W_qkv[:, 0:C][:, cols], W_qkv[:, C:2 * C][:, cols],
             W_qkv[:, 2 * C:3 * C][:, cols]],
            axis=1,
        )
        in_maps.append({
            "xT": np.ascontiguousarray(x[b].T).astype(bf),
            "wqkv": np.ascontiguousarray(w).astype(bf),
            "wproj": np.ascontiguousarray(W_proj[g * DC:(g + 1) * DC, :]).astype(bf),
        })
    return in_maps


def unshard_output(results, b_proj):
    b_proj = np.asarray(b_proj, dtype=np.float32)
    out = np.empty((4, N, C), dtype=np.float32)
    for b in range(4):
        out[b] = results[2 * b]["out"] + results[2 * b + 1]["out"] + b_proj[None, :]
    return out


_NC_CACHE = []


def kernel(x, W_qkv, W_proj, b_proj, trace=False):
    in_maps = shard_inputs(x, W_qkv, W_proj)
    if not _NC_CACHE:
        _NC_CACHE.append(build_program())
    nc = _NC_CACHE[0]
    res = run_bass_kernel_spmd(nc, in_maps, core_ids=list(range(8)), trace=trace)
    out = unshard_output(res.results, b_proj)
    if trace:
        return out, res
    return out
